# revision 16
# baseline (speedup 1.0000x reference)
"""GuidedFilterLayer Trainium2 kernel (8 NeuronCores, batch-sharded).

Math (derived from the reference):
    inputs   = (x+1)/2
    gray     = w0*R + w1*G + w2*B              (on x directly)
    guidance = 0.5*(gray + delta),  delta = mean(x) - mean(gray) + 1
    smoothed = box15(guidance)  (SAME zero pad) = (CB + delta*Wmap)/(225*2)
        CB = colblur15(rowblur15(gray)) un-normalized, Wmap = wr (x) wr
    out      = 0.99*x + (CB + delta*Wmap)*(0.01/225) - 0.01

Design notes:
  * Fixing delta=1 perturbs the output by <1e-4 (tolerance 2e-2): no
    collectives, no mean-reduction chain.
  * x is staged to DRAM pre-scaled by 0.99, in fp16, channel-major
    ([rows, c, w]); output is produced channel-major fp16 and
    unscrambled/cast on CPU.  The kernel is HBM-bound: 3.15MB in +
    3.15MB out per core at ~360 GB/s is ~17.6us; everything else hides
    under the DMA stream.
  * Column blur on TensorE: banded [128,128] matmuls (<=3 row-band
    blocks x 3 channels per chunk, channel weights folded into bands).
  * Row blur on DVE in ONE instruction per chunk using the scan's dual
    op: state = (data0[t] + state) - data1[t] with data1 = the same
    array lagged by 15 computes the sliding 15-wide box sum directly
    (no separate prefix + subtract).  Pad columns (7 lead/tail = beta,
    15 zeros for the lag) make edge windows exact, including the
    -0.01 bias and the Wmap row term.
  * Per-partition bias (0.01/225*wr_col + beta) rides the PSUM->SBUF
    copy on ScalarE.
  * Combines (out = x' + sm broadcast over channels) on DVE, except
    chunks 0/4 on GpSimd to keep DVE under the DMA window.
  * All SBUF tiles are persistent (no pool recycling) to minimize
    semaphore traffic; loads are batched (1,1,2,2,2 chunks).
"""

import numpy as np

B, H, W, C = 16, 512, 512, 3
NCORES = 8
B_LOC = B // NCORES          # 2 images per core
ROWS = B_LOC * H             # 1024 rows per core
FREE = W * C                 # 1536
NCHUNK = ROWS // 128         # 8 chunks of [128, 1536]
MPERIM = H // 128            # 4 row-chunks per image
R_ = 7
K_ = 15
EPS = 0.01
W0, W1, W2 = 0.2989, 0.5870, 0.1140
SCALE_SM = EPS / (K_ * K_)    # 0.01/225
BIAS_SM = -EPS                # -0.01
BETA = BIAS_SM / K_           # per-element bias in the scan input
CMAIN = 1.0 - EPS             # 0.99

# pcsx layout per chunk: [0:15]=0 (lag zeros) | [15:22]=beta (left pad)
# | [22:534]=data | [534:541]=beta (right pad)
LAG = K_                      # 15
PADL = R_                     # 7
PX = LAG + PADL + W + R_      # 541
SMW = PADL + W + R_           # 526 scan outputs; sm[w] = smt[14+w]

_cache = {}


def stage(x):
    """[B,H,W,C] fp32 -> per-core [ROWS, C*W] fp16 channel-major, x0.99."""
    arrs = []
    for i in range(NCORES):
        xc = x[i * B_LOC:(i + 1) * B_LOC]             # [2, H, W, C]
        xc = np.transpose(xc, (0, 1, 3, 2))           # [2, H, C, W]
        arrs.append(np.ascontiguousarray(
            (xc * CMAIN).astype(np.float16).reshape(ROWS, FREE)))
    return arrs


def unstage(res):
    """per-core [ROWS, C*W] fp16 -> [B_LOC,H,W,C] fp32."""
    o = np.asarray(res, dtype=np.float32).reshape(B_LOC, H, C, W)
    return np.transpose(o, (0, 1, 3, 2))


def _band_blocks():
    idx = np.arange(2 * 128)
    band = (np.abs(idx[:, None] - idx[None, :]) <= R_).astype(np.float32)
    bdiag = band[0:128, 0:128]        # kk == mm
    bup = band[0:128, 128:256]        # kk == mm-1  (rows above)
    bdn = band[128:256, 0:128]        # kk == mm+1  (rows below)
    return bdiag, bup, bdn


def _wr_col4():
    i = np.arange(H)
    wr = (np.minimum(i + R_, H - 1) - np.maximum(i - R_, 0) + 1).astype(
        np.float32)
    return wr.reshape(MPERIM, 128).T  # [128, 4]: col mm = wr[128*mm + p]


def _build():
    from contextlib import ExitStack
    from concourse import bass, bacc, tile
    import concourse.mybir as mybir

    f32 = mybir.dt.float32
    f16 = mybir.dt.float16
    Alu = mybir.AluOpType
    Act = mybir.ActivationFunctionType

    nc = bacc.Bacc(
        "TRN2",
        target_bir_lowering=False,
        debug=False,
        enable_asserts=False,
    )

    x_in = nc.dram_tensor("x", [ROWS, FREE], f16, kind="ExternalInput")
    out_d = nc.dram_tensor("out", [ROWS, FREE], f16, kind="ExternalOutput")

    # band blocks scaled by w_c/0.99 (x arrives pre-scaled by 0.99)
    bdiag, bup, bdn = _band_blocks()
    blocks = []
    for blk in (bdiag, bup, bdn):
        for w in (W0, W1, W2):
            blocks.append(blk * (w / CMAIN))
    bands_np = np.concatenate(blocks, axis=1)  # [128, 9*128]
    bands_d = nc.inline_tensor(bands_np.astype(np.float16), name="bands3")
    wrc4_np = (_wr_col4() * SCALE_SM + BETA).astype(np.float32)
    wrc4_d = nc.inline_tensor(np.ascontiguousarray(wrc4_np), name="wrc4")

    with tile.TileContext(nc) as tc, ExitStack() as ctx:
        cp = ctx.enter_context(tc.tile_pool(name="cp", bufs=1))
        pcb = ctx.enter_context(tc.tile_pool(name="pcb", bufs=6, space="PSUM"))
        psw = ctx.enter_context(tc.tile_pool(name="psw", bufs=1, space="PSUM"))

        # ---- persistent tiles ----
        xh = cp.tile([128, NCHUNK * FREE], f16, tag="xh")   # all 8 chunks
        xh3 = xh[:].rearrange("p (t f) -> p t f", f=FREE)
        bsb = cp.tile([128, 9 * 128], f16, tag="bands")
        wrc4 = cp.tile([128, MPERIM], f32, tag="wrc4")
        pcx = cp.tile([128, NCHUNK * PX], f16, tag="pcx")
        pcx3 = pcx[:].rearrange("p (t f) -> p t f", f=PX)
        smt = cp.tile([128, NCHUNK * SMW], f16, tag="smt")
        smt3 = smt[:].rearrange("p (t f) -> p t f", f=SMW)
        ots = cp.tile([128, NCHUNK * FREE], f16, tag="ot")
        ot3 = ots[:].rearrange("p (t f) -> p t f", f=FREE)
        ones = cp.tile([128, 1], f32, tag="ones")

        warm = cp.tile([128, W], f16, tag="warm")
        warmo = cp.tile([128, 1], f32, tag="warmo")
        xb1 = cp.tile([R_, FREE], f16, tag="xb1")   # chunk1 rows 0..6

        # ---- input DMAs ----
        # sync: consts + chunk1.. batches; vector (idle until ~13us):
        # chunk0 channel slices + the 7-row chunk1 head that chunk0's
        # dn-block needs, so chunk0 never waits for the big chunk1 load
        x3 = x_in[:].rearrange("(t p) f -> p t f", p=128)   # [128, 8, 1536]
        nc.sync.dma_start(out=xh3[:, 0:1, :], in_=x3[:, 0:1, :])
        nc.sync.dma_start(out=bsb[:], in_=bands_d[:])
        nc.sync.dma_start(out=wrc4[:], in_=wrc4_d[:])
        nc.sync.dma_start(out=xb1[:], in_=x3[0:R_, 1, :])
        nc.sync.dma_start(out=xh3[:, 1:2, :], in_=x3[:, 1:2, :])
        nc.sync.dma_start(out=xh3[:, 2:4, :], in_=x3[:, 2:4, :])
        nc.sync.dma_start(out=xh3[:, 4:6, :], in_=x3[:, 4:6, :])
        nc.sync.dma_start(out=xh3[:, 6:8, :], in_=x3[:, 6:8, :])

        # ---- pad setup (off critical path; vector is idle early and
        # same-engine order vs the scans makes the pads race-free) ----
        nc.vector.memset(warm[:], 0.0)
        nc.vector.memset(pcx3[:, :, 0:LAG], 0.0)
        nc.vector.memset(pcx3[:, :, LAG:LAG + PADL], float(BETA))
        nc.vector.memset(pcx3[:, :, LAG + PADL + W:PX], float(BETA))
        # PE p-state + activation-table warmup: dummy matmuls on memset
        # data while the first chunk loads, then a tiny ACT reading PSUM
        pcw = psw.tile([128, W], f32, tag="pcw")
        for i in range(6):
            nc.tensor.matmul(out=pcw[:], lhsT=warm[:, 0:128], rhs=warm[:],
                             start=(i == 0), stop=(i == 5))
        nc.scalar.activation(out=warmo[:], in_=pcw[:, 0:1], func=Act.Identity,
                             bias=0.0, scale=1.0)

        # ---- per-chunk pipeline ----
        for t in range(NCHUNK):
            im, mm = divmod(t, MPERIM)
            pc = pcb.tile([128, W], f32, tag="pc")
            ks = [(mm, 0)]
            if mm > 0:
                ks.append((mm - 1, 1))
            if mm < MPERIM - 1:
                ks.append((mm + 1, 2))
            n_mm = len(ks) * 3
            i_mm = 0
            for kk, blk in ks:
                for c in range(3):
                    if t == 0 and blk == 2:
                        # dn-block via the 7-row chunk1 head (xb1):
                        # contraction over just the 7 nonzero band rows
                        lhsT = bsb[0:R_, (blk * 3 + c) * 128:
                                   (blk * 3 + c) * 128 + 128]
                        rhs = xb1[0:R_, c * W:(c + 1) * W]
                    else:
                        lhsT = bsb[:, (blk * 3 + c) * 128:
                                   (blk * 3 + c + 1) * 128]
                        rhs = xh3[:, im * MPERIM + kk, c * W:(c + 1) * W]
                    nc.tensor.matmul(
                        out=pc[:], lhsT=lhsT, rhs=rhs,
                        start=(i_mm == 0), stop=(i_mm == n_mm - 1))
                    i_mm += 1
            # PSUM -> SBUF with scale + per-partition bias; first chunk is
            # split in halves so the scan chain starts earlier, the last
            # chunk so its store starts during the second half
            o3 = ot3[:, t, :].rearrange("p (c w) -> p c w", c=C)
            x3f = xh3[:, t, :].rearrange("p (c w) -> p c w", c=C)
            od3 = out_d[128 * t:128 * (t + 1), :].rearrange(
                "p (c w) -> p c w", c=C)
            if t == 0:
                halves = [(0, 264), (264, W)]
            elif t == NCHUNK - 1:
                halves = [(0, 384), (384, W)]
            else:
                halves = [(0, W)]
            for (a0, a1) in halves:
                nc.scalar.activation(
                    out=pcx3[:, t, LAG + PADL + a0:LAG + PADL + a1],
                    in_=pc[:, a0:a1],
                    func=Act.Identity, bias=wrc4[:, mm:mm + 1],
                    scale=float(SCALE_SM))
                # sliding box15: state = (data0 + state) - data1
                s0 = 0 if a0 == 0 else a0 + R_
                s1 = SMW if a1 == W else a1 + R_
                nc.vector.tensor_tensor_scan(
                    out=smt3[:, t, s0:s1], data0=pcx3[:, t, LAG + s0:LAG + s1],
                    data1=pcx3[:, t, s0:s1],
                    initial=(0.0 if s0 == 0 else smt3[:, t, s0 - 1:s0]),
                    op0=Alu.add, op1=Alu.subtract)
                # combine: out = x' + sm (broadcast over channels)
                j0 = 0 if a0 == 0 else a0 - R_ - 1
                j1 = W if a1 == W else a1 - R_ - 1
                smv = smt3[:, t, LAG - 1 + j0:LAG - 1 + j1]
                nc.vector.tensor_tensor(
                    out=o3[:, :, j0:j1], in0=x3f[:, :, j0:j1],
                    in1=smv.unsqueeze(1).broadcast_to([128, C, j1 - j0]),
                    op=Alu.add)
                if (a0, a1) == (0, W):
                    nc.sync.dma_start(
                        out=out_d[128 * t:128 * (t + 1), :], in_=ot3[:, t, :])
                else:
                    nc.sync.dma_start(out=od3[:, :, j0:j1],
                                      in_=o3[:, :, j0:j1])

    nc.finalize()
    return nc


def _get_nc():
    if "nc" not in _cache:
        _cache["nc"] = _build()
    return _cache["nc"]


def kernel(x):
    from concourse.bass_utils import run_bass_kernel_spmd

    x = np.asarray(x, dtype=np.float32)
    assert x.shape == (B, H, W, C)
    nc = _get_nc()
    in_maps = [{"x": a} for a in stage(x)]
    res = run_bass_kernel_spmd(nc, in_maps, core_ids=list(range(NCORES)))
    out = np.concatenate(
        [unstage(res.results[i]["out"]) for i in range(NCORES)], axis=0)
    return out
